# revision 1
# baseline (speedup 1.0000x reference)
"""Trainium2 Bass kernel for BiLevelRoutingAttention (nn_BiLevelRoutingAttention_66907000537867).

Sharding: one attention head per NeuronCore (8 heads / 8 cores). Each core:
  phase 1: qkv projection for its head (f32 matmuls), producing
           q/k channel-major (bf16), v in padded-image layout (bf16, for the
           lepe depthwise conv folded into the output projection) and v in
           pixel-major layout with a ones column (bf16, for attn@V + softmax
           denominators).
  phase 2: per region (49): QK^T -> exp (ScalarE, scale fused) -> attn@V.
           Softmax normalization: denominators (ones-column matmul output) are
           scatter-DMA'd across partitions, reciprocal'd on DVE, gathered back,
           broadcast via a K=1 matmul and multiplied in.
  phase 3: output projection with lepe folded in: 10 accumulating taps
           (9 shifted dwconv taps with host-folded diag(lepe_w) @ w_o + the
           attention tap), row-tiled 4x across PE via tile_position.

Host: window-ordering of pixels, region routing (top-k is metadata; the mean
commutes exactly with the linear qkv layer), per-head weight slicing, final
sum of per-core partials + constant bias row.
"""

import numpy as np
import ml_dtypes

import concourse.bass as bass
import concourse.bacc as bacc
import concourse.mybir as mybir
import concourse.tile as tile
from concourse.tile import add_dep_helper
from concourse.bass_utils import run_bass_kernel_spmd

F32 = mybir.dt.float32
BF16 = mybir.dt.bfloat16
AF = mybir.ActivationFunctionType

DIM, QK, HEADS, NWIN, TOPK = 256, 256, 8, 7, 4
H = W = 112
P2 = NWIN * NWIN          # 49 regions
W2 = 256                  # pixels per region (16x16)
NPIX = H * W              # 12544
HD = 32                   # per-head dim
SCALE = QK ** (-0.5)      # 1/16
NT = 25                   # pixel tiles: 24x512 + 1x256
PW = 114                  # padded image width
N3 = 448                  # phase-3 pixel tile (4 image rows)

_cache = {}


def _tile_w(t):
    return 512 if t < 24 else 256


def _build(top_idx, debug=False):
    nc = bacc.Bacc()
    xT_d = nc.declare_dram_parameter("xT", [DIM, NPIX], F32, isOutput=False)
    wqkv_d = nc.declare_dram_parameter("wqkv", [DIM, 96], F32, isOutput=False)
    bqkv_d = nc.declare_dram_parameter("bqkv", [96, 1], F32, isOutput=False)
    wt_d = nc.declare_dram_parameter("wt", [HD, 20 * 128], BF16, isOutput=False)
    out_d = nc.declare_dram_parameter("out", [DIM, NPIX], F32, isOutput=True)
    dsc_d = nc.dram_tensor("dscratch", [25, 512], F32)
    dsc2_d = nc.dram_tensor("dscratch2", [25, 512], F32)

    with tile.TileContext(nc) as tc, tc.tile_pool(name="persist", bufs=1) as persist:
        # ---- persistent SBUF ----
        w_sb = persist.tile([128, 192], F32)          # qkv weights, 2 cin chunks
        bqkv_sb = persist.tile([96, 1], F32)
        qk_sb = persist.tile([64, NPIX], BF16)        # rows 0-31 q, 32-63 k
        kx_sb = persist.tile([32, NPIX], BF16)        # k copy at partition base 0
        v_aug = persist.tile([128, 98, 34], BF16)     # pixel-major v + ones col 32 (34-stride keeps rows 4B-aligned)
        v_pix = persist.tile([128, 98 * 32], BF16)    # contiguous transpose staging
        v_pad = persist.tile([128, PW * PW], BF16)    # 4 bands of padded-image v
        out_u = persist.tile([32, NPIX], F32)         # unnormalized attn out (ch-major)
        out_cm = persist.tile([128, NPIX], BF16)      # 4 bands of normalized attn out
        wt_sb = persist.tile([128, 20 * 128], BF16)   # 4 bands of proj stationaries
        ones_sb = persist.tile([1, 32], F32)

        nc.sync.dma_start(out=w_sb[:, 0:96], in_=wqkv_d[0:128, :])
        nc.sync.dma_start(out=w_sb[:, 96:192], in_=wqkv_d[128:256, :])
        nc.sync.dma_start(out=bqkv_sb, in_=bqkv_d[:, :])
        for b in range(4):
            nc.sync.dma_start(out=wt_sb[32 * b:32 * b + 32, :], in_=wt_d[:, :])
        nc.vector.memset(ones_sb, 1.0)
        nc.vector.memset(v_aug[:, :, 32:33], 1.0)
        # zero the padded border (whole tensor; interiors overwritten)
        nc.gpsimd.memset(v_pad, 0.0)

        v_pad_v = v_pad.rearrange("p (r c) -> p r c", c=PW)

        # ---- phase 1: qkv projection ----
        with (
            tc.tile_pool(name="xt", bufs=3) as xtp,
            tc.tile_pool(name="vstage", bufs=4) as vsp,
            tc.tile_pool(name="qkv_ps", bufs=2, space="PSUM") as qkvps,
            tc.tile_pool(name="dum_ps", bufs=2, space="PSUM") as dumps,
        ):
            # this walrus only allows ONE sync wait on a self-loading f32
            # matmul: pre-observe each DMA semaphore with a tiny dummy matmul
            # ordered before the real one so the real matmul needs <=1 wait.
            def observe(aps, dum_pool):
                dum = dum_pool.tile([1, 1], F32, tag="dum")
                last = None
                for ap in aps:
                    d = nc.tensor.matmul(dum, ap[0:1, 0:1], ap[0:1, 0:1],
                                         start=True, stop=True)
                    if last is not None:
                        add_dep_helper(d.ins, last.ins, sync=False)
                    last = d
                return last

            for t in range(NT):
                w = _tile_w(t)
                n0 = 512 * t
                xt0 = xtp.tile([128, w], F32, tag="xt0")
                xt1 = xtp.tile([128, w], F32, tag="xt1")
                nc.sync.dma_start(out=xt0, in_=xT_d[0:128, n0:n0 + w])
                nc.sync.dma_start(out=xt1, in_=xT_d[128:256, n0:n0 + w])
                obs = observe([w_sb, xt0, xt1] if t == 0 else [xt0, xt1], dumps)
                ps = qkvps.tile([96, w], F32, tag="qkv")
                m1 = nc.tensor.matmul(ps, w_sb[:, 0:96], xt0, start=True, stop=False)
                add_dep_helper(m1.ins, obs.ins, sync=False)
                nc.tensor.matmul(ps, w_sb[:, 96:192], xt1, start=False, stop=True)
                # q+k evacuation with bias, f32 -> bf16 (single DVE op keeps the
                # psum-WAR fan-in on one engine semaphore)
                nc.vector.tensor_scalar_add(qk_sb[:, n0:n0 + w], ps[0:64, :],
                                            bqkv_sb[0:64, 0:1])
                # v evacuation to bf16 staging (partitions 64-95)
                vs = vsp.tile([96, w], BF16, tag="vs")
                nc.vector.tensor_scalar_add(vs[64:96, :], ps[64:96, :],
                                            bqkv_sb[64:96, 0:1])
                # v -> padded image layout (band 2 = partitions 64-95)
                vsv = vs.rearrange("p (a b c) -> p a b c", b=16, c=16)
                for wi in range(w // 256):
                    win = 2 * t + wi
                    wr, wc = divmod(win, NWIN)
                    nc.vector.tensor_copy(
                        v_pad_v[64:96, 16 * wr + 1:16 * wr + 17,
                                16 * wc + 1:16 * wc + 17],
                        vsv[64:96, wi, :, :])
                # v -> pixel-major staging via DMA xbar transpose; one
                # 128-col tile per call, 64B-aligned contiguous destinations
                for ci in range(w // 128):
                    c = 4 * t + ci
                    nc.sync.dma_start_transpose(
                        out=v_pix[:, 32 * c:32 * (c + 1)],
                        in_=vs[64:96, 128 * ci:128 * (ci + 1)])

            # interleave v into the 34-stride v_aug slots (col 32 stays ones)
            nc.vector.tensor_copy(v_aug[:, :, 0:32],
                                  v_pix.rearrange("p (c j) -> p c j", j=32))
            # k copy to partition base 0; v_pad band replication
            nc.sync.dma_start(out=kx_sb[:, :], in_=qk_sb[32:64, :])
            for b in (0, 1, 3):
                nc.sync.dma_start(out=v_pad[32 * b:32 * b + 32, :],
                                  in_=v_pad[64:96, :])

        tc.strict_bb_all_engine_barrier()

        # ---- phase 2: attention ----
        with (
            tc.tile_pool(name="attnT_ps", bufs=2, space="PSUM") as atps,
            tc.tile_pool(name="outT_ps", bufs=1, space="PSUM") as otps,
            tc.tile_pool(name="bc_ps", bufs=1, space="PSUM") as bcps,
            tc.tile_pool(name="expT", bufs=4) as expp,
            tc.tile_pool(name="dstage", bufs=4) as dsp,
            tc.tile_pool(name="dum2_ps", bufs=2, space="PSUM") as dum2,
        ):
            def observe2(aps):
                dum = dum2.tile([1, 1], F32, tag="dum2")
                last = None
                for ap in aps:
                    d = nc.tensor.matmul(dum, ap[0:1, 0:1], ap[0:1, 0:1],
                                         start=True, stop=True)
                    if last is not None:
                        add_dep_helper(d.ins, last.ins, sync=False)
                    last = d
                return last

            dst = None
            for r in range(P2):
                chunks = [2 * g + jj for g in top_idx[r] for jj in (0, 1)]
                q_ap = qk_sb[0:32, W2 * r:W2 * (r + 1)]
                exs = []
                for half in range(2):
                    at = atps.tile([128, 1024], F32, tag="at")
                    for j4 in range(4):
                        c = chunks[4 * half + j4]
                        nc.tensor.matmul(at[:, 256 * j4:256 * (j4 + 1)],
                                         kx_sb[:, 128 * c:128 * (c + 1)],
                                         q_ap, start=True, stop=True)
                    ex = expp.tile([128, 1024], BF16, tag="ex")
                    nc.scalar.activation(ex, at, AF.Exp, scale=SCALE)
                    exs.append(ex)
                outT = otps.tile([33, W2], F32, tag="ot")
                for j in range(8):
                    nc.tensor.matmul(outT, v_aug[:, chunks[j], 0:33],
                                     exs[j // 4][:, 256 * (j % 4):256 * (j % 4 + 1)],
                                     start=(j == 0), stop=(j == 7))
                nc.vector.tensor_copy(out_u[:, W2 * r:W2 * (r + 1)], outT[0:32, :])
                # stash denominators (psum row 32 -> sbuf partition 32)
                if r % 2 == 0:
                    dst = dsp.tile([33, 512], F32, tag="dst")
                nc.vector.tensor_copy(dst[32:33, 256 * (r % 2):256 * (r % 2) + 256],
                                      outT[32:33, :])
                if r % 2 == 1 or r == P2 - 1:
                    pw = 512 if r % 2 == 1 else 256
                    p0 = 512 * (r // 2)
                    pi = r // 2
                    nj = pw // 128
                    # scatter across partitions (via DRAM), reciprocal, gather back
                    nc.sync.dma_start(out=dsc_d[pi, 0:pw], in_=dst[32:33, 0:pw])
                    dp = dsp.tile([128, 4], F32, tag="dp")
                    nc.sync.dma_start(
                        out=dp[:, 0:nj],
                        in_=dsc_d[pi, 0:pw].rearrange("(j q) -> q j", q=128))
                    dpr = dsp.tile([128, 4], F32, tag="dpr")
                    nc.vector.reciprocal(dpr[:, 0:nj], dp[:, 0:nj])
                    nc.sync.dma_start(
                        out=dsc2_d[pi, 0:pw].rearrange("(j q) -> q j", q=128),
                        in_=dpr[:, 0:nj])
                    dr = dsp.tile([1, 512], F32, tag="dr")
                    nc.sync.dma_start(out=dr[0:1, 0:pw], in_=dsc2_d[pi, 0:pw])
                    bc = bcps.tile([32, 512], F32, tag="bc")
                    obs = observe2([dr])
                    mb = nc.tensor.matmul(bc[:, 0:pw], ones_sb[:, :], dr[0:1, 0:pw],
                                          start=True, stop=True)
                    add_dep_helper(mb.ins, obs.ins, sync=False)
                    nc.vector.tensor_mul(out_cm[0:32, p0:p0 + pw],
                                         out_u[:, p0:p0 + pw], bc[:, 0:pw])

            # out_cm band replication
            for b in (1, 2, 3):
                nc.sync.dma_start(out=out_cm[32 * b:32 * b + 32, :],
                                  in_=out_cm[0:32, :])

        tc.strict_bb_all_engine_barrier()

        # ---- phase 3: output projection + folded lepe ----
        out_cm_w = out_cm.rearrange("p (w a b) -> p w a b", a=16, b=16)
        with (
            tc.tile_pool(name="o_ps", bufs=4, space="PSUM") as ops,
            tc.tile_pool(name="osb", bufs=4) as osbp,
        ):
            for n in range(28):
                b = n % 4
                sl = slice(32 * b, 32 * b + 32)
                tp = (32 * b, 0)
                for hh in range(2):
                    acc = ops.tile([128, N3], F32, tag="acc")
                    for t in range(9):
                        dy, dx = divmod(t, 3)
                        rhs = v_pad_v[sl, 4 * n + dy:4 * n + dy + 4, dx:dx + 112]
                        nc.tensor.matmul(acc, wt_sb[sl, 128 * (2 * t + hh):
                                                    128 * (2 * t + hh + 1)],
                                         rhs, start=(t == 0), stop=False,
                                         tile_position=tp)
                    wr_, py0 = n // 4, (4 * n) % 16
                    rhs = out_cm_w[sl, 7 * wr_:7 * wr_ + 7, py0:py0 + 4, :]
                    rhs = rhs.rearrange("p w a b -> p a w b")
                    nc.tensor.matmul(acc, wt_sb[sl, 128 * (18 + hh):128 * (19 + hh)],
                                     rhs, start=False, stop=True, tile_position=tp)
                    ev = osbp.tile([128, N3], F32, tag="ev")
                    if n % 2 == 0:
                        nc.vector.tensor_copy(ev, acc)
                    else:
                        nc.scalar.copy(ev, acc)
                    nc.sync.dma_start(
                        out=out_d[128 * hh:128 * (hh + 1), N3 * n:N3 * (n + 1)],
                        in_=ev)

        if debug:
            tc.strict_bb_all_engine_barrier()
            dbg = {
                "dbg_qk": qk_sb, "dbg_kx": kx_sb, "dbg_vaug": v_aug,
                "dbg_vpad": v_pad, "dbg_outu": out_u, "dbg_outcm": out_cm,
            }
            for name, t in dbg.items():
                sh = [t.shape[0], int(np.prod(t.shape[1:]))]
                d = nc.declare_dram_parameter(name, sh, t.dtype, isOutput=True)
                nc.sync.dma_start(out=d[:, :], in_=t.rearrange(
                    "p ... -> p (...)") if len(t.shape) > 2 else t[:, :])
            dd = nc.declare_dram_parameter("dbg_dr", [25, 512], F32, isOutput=True)
            nc.sync.dma_start(out=dd[:, :], in_=dsc2_d[:, :])
    nc.compile()
    return nc


def _host_prep(x, w_qkv, b_qkv):
    xw = x.reshape(NWIN, 16, NWIN, 16, DIM).transpose(0, 2, 1, 3, 4)
    xw = np.ascontiguousarray(xw.reshape(NPIX, DIM))
    xT = np.ascontiguousarray(xw.T)
    xmean = xw.reshape(P2, W2, DIM).mean(1)
    q_win = xmean @ w_qkv[:, :QK] + b_qkv[:QK]
    k_win = xmean @ w_qkv[:, QK:2 * QK] + b_qkv[QK:2 * QK]
    logit = (q_win * SCALE) @ k_win.T
    top_idx = np.argsort(-logit, axis=-1, kind="stable")[:, :TOPK]
    return xT, top_idx


def _in_maps(x, w_qkv, b_qkv, w_o, lepe_w):
    xT, top_idx = _host_prep(x[0], w_qkv, b_qkv)
    lw = lepe_w[:, :, 0, :]  # [3,3,256]
    maps = []
    for h in range(HEADS):
        sl = slice(h * HD, (h + 1) * HD)
        wqkv_h = np.concatenate(
            [w_qkv[:, :QK][:, sl], w_qkv[:, QK:2 * QK][:, sl],
             w_qkv[:, 2 * QK:][:, sl]], axis=1)
        bqkv_h = np.concatenate(
            [b_qkv[:QK][sl], b_qkv[QK:2 * QK][sl], b_qkv[2 * QK:][sl]])
        w_o_h = w_o[sl, :]  # [32, 256]
        blocks = []
        for t in range(9):
            dy, dx = divmod(t, 3)
            wt_full = lw[dy, dx, sl][:, None] * w_o_h
            blocks += [wt_full[:, 0:128], wt_full[:, 128:256]]
        blocks += [w_o_h[:, 0:128], w_o_h[:, 128:256]]
        wt_h = np.concatenate(blocks, axis=1).astype(ml_dtypes.bfloat16)
        maps.append({
            "xT": xT,
            "wqkv": np.ascontiguousarray(wqkv_h),
            "bqkv": np.ascontiguousarray(bqkv_h[:, None]),
            "wt": np.ascontiguousarray(wt_h),
        })
    return maps, top_idx


def kernel(x, w_qkv, b_qkv, w_o, b_o, lepe_w, lepe_b):
    x = np.asarray(x, np.float32)
    w_qkv = np.asarray(w_qkv, np.float32)
    b_qkv = np.asarray(b_qkv, np.float32)
    w_o = np.asarray(w_o, np.float32)
    b_o = np.asarray(b_o, np.float32)
    lepe_w = np.asarray(lepe_w, np.float32)
    lepe_b = np.asarray(lepe_b, np.float32)

    maps, top_idx = _in_maps(x, w_qkv, b_qkv, w_o, lepe_w)
    key = top_idx.tobytes()
    if key not in _cache:
        _cache[key] = _build(top_idx)
    nc = _cache[key]

    res = run_bass_kernel_spmd(nc, maps, list(range(HEADS))).results
    total = np.zeros((DIM, NPIX), np.float32)
    for h in range(HEADS):
        total += np.asarray(res[h]["out"], np.float32)
    b_all = lepe_b @ w_o + b_o
    out = total.T + b_all
    return out.reshape(1, H, W, DIM).astype(np.float32)

